# revision 23
# baseline (speedup 1.0000x reference)
"""Trainium2 Bass kernel: GQA causal self-attention with ALiBi.

Problem: B=4, T=2048, C=2048, 16 Q heads / 4 KV heads, head_dim=128, fp32.

Sharding (8 cores): DP2 x TP4. Core c = (bg, g) with bg = c//4 (batches
2bg, 2bg+1), g = c%4 (KV group g = Q heads 4g..4g+3 + KV head g). The
reference's ALiBi slope is constant within a KV group (slopes[h//4]), so
each core has a single slope. Host feeds x^T per batch (transpose-free
dataflow on chip) and sums the 4 partial Wo outputs per batch.

v2 design (vs baseline):
- Head-batched attention: the 4 Q heads of a core share one KV head and
  one slope, so one [128,512] matmul computes scores for all 4 heads x a
  128-query chunk against one 128-key chunk. ALiBi truncates attention
  to a 2-chunk window (1 prior + diagonal); dropped keys have relative
  weight < e^-32.
- ALiBi+mask applied as p = exp(s) * expf (expf precomputed, 0 at masked
  entries) - one ACT op + one bf16 2x-mode DVE op per chunk.
- Softmax denominator: ones-matmul partition reduction (broadcast across
  partitions) accumulated on PE, then reciprocal_approx_fast (single
  custom-DVE op, ~18 bits) - no [1,512] one-lane reciprocal, no
  PE->ACT->DVE->PE round trip.
- All host-side layouts pre-transposed so every DMA is a straight copy.
- bf16 output partials (halves out-DMA); host sums in fp32.
"""

import math
from contextlib import ExitStack

import ml_dtypes
import numpy as np

import concourse.bass as bass
import concourse.mybir as mybir
import concourse.tile as tile
from concourse import bacc
from concourse.bass_utils import run_bass_kernel_spmd

B, T, C = 4, 2048, 2048
HD = 128          # head dim
HPC = 4           # Q heads per core
QB = 512          # projection block (free dim)
KC = 128          # key/query chunk for attention
NQB = T // QB     # 4
NCC = C // 128    # 16 contraction chunks for projections

F32 = mybir.dt.float32
BF16 = mybir.dt.bfloat16
EXP = mybir.ActivationFunctionType.Exp

_CACHE = {}


def build_kernel():
    nc = bacc.Bacc(
        "TRN2",
        target_bir_lowering=False,
        debug=False,
        enable_asserts=False,
        num_devices=8,
    )
    xh_d = nc.dram_tensor("xh", [2, 128, NCC, T], BF16, kind="ExternalInput").ap()
    wq_d = nc.dram_tensor("wq", [128, NCC, HPC * HD], BF16, kind="ExternalInput").ap()
    wk_d = nc.dram_tensor("wk", [128, NCC, HD], BF16, kind="ExternalInput").ap()
    wv_d = nc.dram_tensor("wv", [128, NCC, HD], BF16, kind="ExternalInput").ap()
    wo_d = nc.dram_tensor("wo", [128, HPC, C], BF16, kind="ExternalInput").ap()
    ef_d = nc.dram_tensor("expf", [128, 2, QB], BF16, kind="ExternalInput").ap()
    id_d = nc.dram_tensor("ident", [128, 128], BF16, kind="ExternalInput").ap()
    on_d = nc.dram_tensor("onesc", [128, 128], BF16, kind="ExternalInput").ap()
    outT = nc.dram_tensor("outT", [2, 128, 16, T], BF16, kind="ExternalOutput").ap()

    with ExitStack() as ctx:
        tc = ctx.enter_context(tile.TileContext(nc))
        ctx.enter_context(
            nc.allow_low_precision(reason="bf16 matmuls; fp32 accumulate")
        )

        consts = ctx.enter_context(tc.tile_pool(name="consts", bufs=1))
        xpool = ctx.enter_context(tc.tile_pool(name="xpool", bufs=2))
        qpool = ctx.enter_context(tc.tile_pool(name="qpool", bufs=2))
        vtpool = ctx.enter_context(tc.tile_pool(name="vtpool", bufs=2))
        epool = ctx.enter_context(tc.tile_pool(name="epool", bufs=4))
        ppool = ctx.enter_context(tc.tile_pool(name="ppool", bufs=4))
        bcpool = ctx.enter_context(tc.tile_pool(name="bcpool", bufs=3))
        ypool = ctx.enter_context(tc.tile_pool(name="ypool", bufs=2))
        opool = ctx.enter_context(tc.tile_pool(name="opool", bufs=3))

        # 8 PSUM banks: proj 3 (the O-proj bank-reuse period must exceed the
        # ~1.9us evacuation chain), s 2, y 2, cs 1 (V-transposes share the cs
        # slot - temporally disjoint from attention's colsum use).
        ps_proj = ctx.enter_context(tc.tile_pool(name="ps_proj", bufs=3, space="PSUM"))
        ps_s = ctx.enter_context(tc.tile_pool(name="ps_s", bufs=2, space="PSUM"))
        ps_y = ctx.enter_context(tc.tile_pool(name="ps_y", bufs=2, space="PSUM"))
        ps_cs = ctx.enter_context(tc.tile_pool(name="ps_cs", bufs=1, space="PSUM"))

        # resident weights / constants (straight DMA copies, host pre-layouts).
        # wq is loaded in 4 cc-slices so the first Q matmuls can start after
        # ~128 KiB instead of the full 7 MiB preload (reads inside the tile
        # depend only on the covering slice-DMA).
        ident = consts.tile([128, 128], BF16, tag="ident")
        nc.sync.dma_start(ident, id_d)
        ones = consts.tile([128, 128], BF16, tag="ones")
        nc.sync.dma_start(ones, on_d)
        ef_sb = consts.tile([128, 2, QB], BF16, tag="ef")
        nc.sync.dma_start(ef_sb, ef_d)
        # Startup DMA serialization: concurrent transfers share bandwidth
        # round-robin, so an ungated burst makes EVERY transfer finish late
        # (~16us). A 1-element copy into each destination (reading the tail
        # of the previous slice) adds a WAW dep that delays the DMA issue,
        # pipelining the burst: first wq/x slice lands ~4us in and the Q
        # matmuls consume slices as they arrive.
        wq_sb = consts.tile([128, NCC, HPC * HD], BF16, tag="wq")
        xin0 = xpool.tile([128, NCC, QB], BF16, tag="x", name="xin0")
        for sl in range(4):
            nc.sync.dma_start(
                wq_sb[:, 4 * sl:4 * sl + 4, :], wq_d[:, 4 * sl:4 * sl + 4, :]
            )
            nc.sync.dma_start(
                xin0[:, 4 * sl:4 * sl + 4, :], xh_d[0, :, 4 * sl:4 * sl + 4, 0:QB]
            )
        wk_sb = consts.tile([128, NCC, HD], BF16, tag="wk")
        nc.sync.dma_start(wk_sb, wk_d)
        wv_sb = consts.tile([128, NCC, HD], BF16, tag="wv")
        nc.sync.dma_start(wv_sb, wv_d)
        # wo (and the second x strip below) have ~40us of slack: a WAW gate
        # (1-elem copy reading the first strip's tail) keeps them out of the
        # startup burst, which shares DMA bandwidth round-robin.
        g3 = xin0[0:1, 15:16, 0:1]
        wo_sb = consts.tile([128, HPC, C], BF16, tag="wo")
        nc.vector.tensor_copy(wo_sb[0:1, 0:1, 0:1], g3)
        nc.sync.dma_start(wo_sb, wo_d)

        # full-batch K^T / V in SBUF (4 KiB/partition each): no ring, no WAR
        kt = [
            consts.tile([128, 16, KC], BF16, tag=f"kt{i}", name=f"kt{i}")
            for i in range(2)
        ]
        vv = [
            consts.tile([128, 16, HD], BF16, tag=f"vv{i}", name=f"vv{i}")
            for i in range(2)
        ]

        def emit_oproj(b, t0, y_sb):
            o_sb = opool.tile([128, 16, QB], BF16, tag="o", name="o_sb")
            for co in range(16):
                ps = ps_proj.tile([128, QB], F32, tag="proj", name="ps")
                for hc in range(HPC):
                    nc.tensor.matmul(
                        ps,
                        lhsT=wo_sb[:, hc, co * 128:(co + 1) * 128],
                        rhs=y_sb[:, hc, :, :],
                        start=(hc == 0),
                        stop=(hc == HPC - 1),
                    )
                nc.any.tensor_copy(o_sb[:, co, :], ps)
                if co % 4 == 3:
                    nc.sync.dma_start(
                        outT[b, :, co - 3:co + 1, t0:t0 + QB],
                        o_sb[:, co - 3:co + 1, :],
                    )

        # O-projection is software-pipelined one block behind so the next
        # block's projections fill the PE while the softmax-normalization
        # tail (cs->recip->y-mul) of this block resolves.
        pending_o = None
        for b in range(2):
            for tb in range(NQB):
                t0 = tb * QB
                if b == 0 and tb == 0:
                    xin = xin0
                else:
                    xin = xpool.tile([128, NCC, QB], BF16, tag="x")
                    if b == 0 and tb == 1:
                        # keep the second strip's prefetch out of the
                        # startup burst (it has ~40us of slack)
                        nc.vector.tensor_copy(xin[0:1, 0:1, 0:1], g3)
                    for sl in range(4):
                        nc.sync.dma_start(
                            xin[:, 4 * sl:4 * sl + 4, :],
                            xh_d[b, :, 4 * sl:4 * sl + 4, t0:t0 + QB],
                        )

                # ---- Q projection ----
                # First block runs cc-outer with 4 parallel accumulators so
                # the matmuls consume wq/x slices as the DMAs land instead of
                # waiting for the whole 4 MiB preload.
                qT = qpool.tile([128, HPC, QB], BF16, tag="q")
                if b == 0 and tb == 0:
                    accs = [
                        ps_proj.tile([128, QB], F32, tag="proj", name="qa0"),
                        ps_proj.tile([128, QB], F32, tag="proj", name="qa1"),
                        ps_s.tile([128, QB], F32, tag="s", name="qa2"),
                        ps_s.tile([128, QB], F32, tag="s", name="qa3"),
                    ]
                    for cc in range(NCC):
                        for h in range(HPC):
                            nc.tensor.matmul(
                                accs[h],
                                lhsT=wq_sb[:, cc, h * HD:(h + 1) * HD],
                                rhs=xin[:, cc, :],
                                start=(cc == 0),
                                stop=(cc == NCC - 1),
                            )
                    for h in range(HPC):
                        nc.any.tensor_copy(qT[:, h, :], accs[h])
                else:
                    for h in range(HPC):
                        ps = ps_proj.tile([128, QB], F32, tag="proj")
                        for cc in range(NCC):
                            nc.tensor.matmul(
                                ps,
                                lhsT=wq_sb[:, cc, h * HD:(h + 1) * HD],
                                rhs=xin[:, cc, :],
                                start=(cc == 0),
                                stop=(cc == NCC - 1),
                            )
                        nc.any.tensor_copy(qT[:, h, :], ps)

                # ---- K projection -> K^T chunks ----
                ps = ps_proj.tile([128, 4, KC], F32, tag="proj")
                for cc in range(NCC):
                    nc.tensor.matmul(
                        ps, lhsT=wk_sb[:, cc, :], rhs=xin[:, cc, :],
                        start=(cc == 0), stop=(cc == NCC - 1),
                    )
                nc.any.tensor_copy(kt[b][:, 4 * tb:4 * tb + 4, :], ps)

                # ---- V projection -> transpose to [k, d] ----
                ps = ps_proj.tile([128, QB], F32, tag="proj")
                for cc in range(NCC):
                    nc.tensor.matmul(
                        ps, lhsT=wv_sb[:, cc, :], rhs=xin[:, cc, :],
                        start=(cc == 0), stop=(cc == NCC - 1),
                    )
                vt = vtpool.tile([128, QB], BF16, tag="vt")
                nc.any.tensor_copy(vt, ps)
                for kc in range(4):
                    tp = ps_cs.tile([128, KC], BF16, tag="cs", name="tp")
                    nc.tensor.transpose(tp, vt[:, kc * KC:(kc + 1) * KC], ident)
                    nc.any.tensor_copy(vv[b][:, 4 * tb + kc, :], tp)

                # ---- attention (all 4 heads per matmul) ----
                y_sb = ypool.tile([128, HPC, 4, KC], BF16, tag="y")
                for qc in range(4):
                    gq = 4 * tb + qc
                    kbs = [k for k in (gq - 1, gq) if k >= 0]
                    y_ps = ps_y.tile([128, HPC, KC], F32, tag="y")
                    cs_ps = ps_cs.tile([128, HPC, KC], F32, tag="cs")
                    for i, kb in enumerate(kbs):
                        m = kb - gq + 1
                        s_ps = ps_s.tile([128, QB], F32, tag="s")
                        nc.tensor.matmul(
                            s_ps,
                            lhsT=kt[b][:, kb, :],
                            rhs=qT[:, :, qc * KC:(qc + 1) * KC],
                        )
                        e = epool.tile([128, QB], BF16, tag="e")
                        nc.scalar.activation(e, s_ps, EXP)
                        p = ppool.tile([128, QB], BF16, tag="p")
                        nc.vector.tensor_mul(p, e, ef_sb[:, m, :])
                        last = i == len(kbs) - 1
                        # cs before y: the reciprocal waits on cs's stop, so
                        # issuing it first starts the norm tail one MM earlier
                        nc.tensor.matmul(
                            cs_ps, lhsT=ones, rhs=p,
                            start=(i == 0), stop=last,
                        )
                        nc.tensor.matmul(
                            y_ps, lhsT=vv[b][:, kb, :], rhs=p,
                            start=(i == 0), stop=last,
                        )
                    bc = bcpool.tile([128, HPC, KC], F32, tag="bc")
                    nc.vector.reciprocal_approx_fast(bc, cs_ps)
                    nc.vector.tensor_mul(y_sb[:, :, qc, :], y_ps, bc)

                if b == 0 and tb == 0:
                    nc.sync.dma_start(wo_sb, wo_d)

                # ---- output projection (previous block) ----
                if pending_o is not None:
                    emit_oproj(*pending_o)
                pending_o = (b, t0, y_sb)
        emit_oproj(*pending_o)

    nc.compile()
    return nc


def make_expf(sigma):
    """expf[p, m, h*128+f] = exp(sigma*((m-1)*128 + p - f)), zeroed where
    the key is after the query (causal); m=0 prior chunk, m=1 diagonal.
    Layout [128, 2, QB] matches the SBUF tile partition-first."""
    p = np.arange(KC, dtype=np.float64)[:, None]
    f = np.arange(KC, dtype=np.float64)[None, :]
    out = np.zeros((2, KC, QB), np.float32)
    for m in range(2):
        o = (m - 1) * 128
        blk = np.exp(sigma * (o + p - f))
        blk[p + o > f] = 0.0
        out[m] = np.tile(blk.astype(np.float32), (1, HPC))
    return np.ascontiguousarray(out.transpose(1, 0, 2)).astype(ml_dtypes.bfloat16)


def kernel(x, Wq, Wk, Wv, Wo):
    import os
    import time

    dbg = os.environ.get("KERNEL_DEBUG") == "1"
    t0 = time.time()

    def tick(msg):
        nonlocal t0
        if dbg:
            print(f"[kernel] {msg}: {time.time() - t0:.2f}s", flush=True)
        t0 = time.time()

    x = np.ascontiguousarray(np.asarray(x, np.float32))
    Wq = np.ascontiguousarray(np.asarray(Wq, np.float32))
    Wk = np.ascontiguousarray(np.asarray(Wk, np.float32))
    Wv = np.ascontiguousarray(np.asarray(Wv, np.float32))
    Wo = np.ascontiguousarray(np.asarray(Wo, np.float32))

    tick("input prep")
    if "nc" not in _CACHE:
        _CACHE["nc"] = build_kernel()
        tick("build_kernel")
    nc = _CACHE["nc"]

    s = 1.0 / math.sqrt(HD)
    slopes = [2.0 ** -0.5, 0.5, 2.0 ** -1.5, 0.25]
    BF = ml_dtypes.bfloat16
    ident = np.eye(128, dtype=BF)
    onesc = np.ones((128, 128), dtype=BF)

    # xh[i, p, cc, t] = x[2bg+i, t, cc*128+p]  (shared within a batch group)
    xh_by_bg = []
    for bg in range(2):
        xh = np.stack([
            np.ascontiguousarray(
                x[2 * bg + i].T.reshape(NCC, 128, T).transpose(1, 0, 2)
            ) for i in range(2)
        ]).astype(BF)
        xh_by_bg.append(xh)

    in_maps = []
    for c in range(8):
        bg, g = c // 4, c % 4
        in_maps.append({
            "xh": xh_by_bg[bg],
            "wq": np.ascontiguousarray(
                (Wq[:, g * 512:(g + 1) * 512] * s)
                .reshape(NCC, 128, 512).transpose(1, 0, 2)).astype(BF),
            "wk": np.ascontiguousarray(
                Wk[:, g * HD:(g + 1) * HD]
                .reshape(NCC, 128, HD).transpose(1, 0, 2)).astype(BF),
            "wv": np.ascontiguousarray(
                Wv[:, g * HD:(g + 1) * HD]
                .reshape(NCC, 128, HD).transpose(1, 0, 2)).astype(BF),
            "wo": np.ascontiguousarray(
                Wo[g * 512:(g + 1) * 512, :]
                .reshape(HPC, 128, C).transpose(1, 0, 2)).astype(BF),
            "expf": make_expf(slopes[g]),
            "ident": ident,
            "onesc": onesc,
        })

    tick("in_maps prep")
    res = run_bass_kernel_spmd(nc, in_maps, core_ids=list(range(8)))
    tick("device run")
    out = np.zeros((B, T, C), np.float32)
    for c in range(8):
        bg, g = c // 4, c % 4
        oT = res.results[c]["outT"]
        for i in range(2):
            out[2 * bg + i] += (
                oT[i].transpose(2, 1, 0).reshape(T, C).astype(np.float32)
            )
    tick("gather")
    return out
